# revision 1
# baseline (speedup 1.0000x reference)
"""Trainium2 Bass kernel for the Flux_Kernels 5-point Dirichlet stencil.

out[i,j] = D*s0*(u[i-1,j] + u[i+1,j] + u[i,j-1] + u[i,j+1]) + 4*D*s1*u[i,j]
with out-of-range neighbors replaced by dirichlet_val[{0,1,2,3}].

Strategy: pad u with the Dirichlet constants into S [4098, 4098] on the host,
shard along rows: core k gets S[512k : 512k+514] (1-row halo each side baked
into the slab). On each core, tiles of 128 consecutive padded rows are
processed with partition p <-> padded row r0+p:
  - TensorE: tridiagonal matmul W.T @ tile -> PSUM[p] = a*up + c*ctr + a*down
    centered at padded row r0+p (rows 0 and 127 are incomplete and discarded)
  - VectorE: lr[p] = tile[p, j] + tile[p, j+2]  (left+right sums)
  - VectorE: o[p] = (lr[p] * a) + PSUM[p]       (fused scalar_tensor_tensor)
  - output DMA stores partitions 1..126 -> 126 output rows per tile; the
    DMA absorbs the one-row shift that compute engines cannot express.
Consecutive tiles overlap by 2 rows; all scalars (a = D*s0, c = 4*D*s1,
weight matrices) are computed on the host from the runtime inputs.
"""

import sys

import numpy as np

if "/opt/trn_rl_repo" not in sys.path:
    sys.path.insert(0, "/opt/trn_rl_repo")

NX, NY = 4096, 4096
N_CORES = 8
ROWS_PER_CORE = NX // N_CORES          # 512
SLAB_ROWS = ROWS_PER_CORE + 2          # 514
PAD_COLS = NY + 2                      # 4098
TILE_OUT = 126                         # output rows per full tile
FULL_TILES = ROWS_PER_CORE // TILE_OUT  # 4
LAST_OUT = ROWS_PER_CORE - FULL_TILES * TILE_OUT  # 8
LAST_IN = LAST_OUT + 2                 # 10
PSUM_HALF = 2048                       # free-dim columns per PSUM tile
MM_N = 512                             # matmul moving free dim (1 PSUM bank)

_CACHE: dict = {}


def _build_nc():
    import concourse.bass as bass
    import concourse.mybir as mybir
    from concourse import bacc
    from concourse.tile import TileContext

    f32 = mybir.dt.float32
    add = mybir.AluOpType.add
    mult = mybir.AluOpType.mult

    nc = bacc.Bacc(None, target_bir_lowering=False)
    s_in = nc.dram_tensor("s_in", (SLAB_ROWS, PAD_COLS), f32, kind="ExternalInput")
    w_main = nc.dram_tensor("w_main", (128, 128), f32, kind="ExternalInput")
    w_last = nc.dram_tensor("w_last", (LAST_IN, LAST_IN), f32, kind="ExternalInput")
    coef = nc.dram_tensor("coef", (128, 1), f32, kind="ExternalInput")
    out = nc.dram_tensor("out", (ROWS_PER_CORE, NY), f32, kind="ExternalOutput")

    n_tiles = FULL_TILES + 1

    with TileContext(nc) as tc:
        with (
            tc.tile_pool(name="const", bufs=1) as cpool,
            tc.tile_pool(name="inp", bufs=3) as ipool,
            tc.tile_pool(name="lrp", bufs=2) as lpool,
            tc.tile_pool(name="op", bufs=2) as opool,
            tc.tile_pool(name="psum", bufs=2, space=bass.MemorySpace.PSUM) as ppool,
        ):
            w_t = cpool.tile([128, 128], f32)
            nc.sync.dma_start(out=w_t[:], in_=w_main[:])
            w5_t = cpool.tile([LAST_IN, LAST_IN], f32)
            nc.sync.dma_start(out=w5_t[:], in_=w_last[:])
            coef_t = cpool.tile([128, 1], f32)
            nc.sync.dma_start(out=coef_t[:], in_=coef[:])

            for t in range(n_tiles):
                r0 = TILE_OUT * t
                ri = 128 if t < FULL_TILES else LAST_IN
                ro = TILE_OUT if t < FULL_TILES else LAST_OUT
                wt = w_t if t < FULL_TILES else w5_t

                in_t = ipool.tile([128, PAD_COLS], f32, tag="in")
                nc.sync.dma_start(out=in_t[:ri], in_=s_in[r0 : r0 + ri, :])

                lr_t = lpool.tile([128, NY], f32, tag="lr")
                nc.vector.tensor_add(
                    out=lr_t[:ri], in0=in_t[:ri, 0:NY], in1=in_t[:ri, 2 : NY + 2]
                )

                o_t = opool.tile([128, NY], f32, tag="o")
                for h in range(NY // PSUM_HALF):
                    ps = ppool.tile([128, PSUM_HALF], f32, tag="ps")
                    for q in range(PSUM_HALF // MM_N):
                        cc = h * PSUM_HALF + q * MM_N
                        nc.tensor.matmul(
                            ps[:ri, q * MM_N : (q + 1) * MM_N],
                            wt[:ri, :ri],
                            in_t[:ri, 1 + cc : 1 + cc + MM_N],
                            start=True,
                            stop=True,
                        )
                    nc.vector.scalar_tensor_tensor(
                        out=o_t[:ri, h * PSUM_HALF : (h + 1) * PSUM_HALF],
                        in0=lr_t[:ri, h * PSUM_HALF : (h + 1) * PSUM_HALF],
                        scalar=coef_t[:ri],
                        in1=ps[:ri, :],
                        op0=mult,
                        op1=add,
                    )
                nc.sync.dma_start(
                    out=out[r0 : r0 + ro, :], in_=o_t[1 : 1 + ro, :]
                )

    nc.compile()
    return nc


def _get_nc():
    if "nc" not in _CACHE:
        _CACHE["nc"] = _build_nc()
    return _CACHE["nc"]


def _tridiag(n, a, c):
    w = np.zeros((n, n), dtype=np.float32)
    i = np.arange(n)
    w[i, i] = c
    w[i[:-1], i[1:]] = a  # k = m-1 (up neighbor)
    w[i[1:], i[:-1]] = a  # k = m+1 (down neighbor)
    return w


def kernel(u_main, u_coupled=None, D_eff=None, dirichlet_val=None, stencil=None,
           t=None, **_ignored):
    u = np.asarray(u_main, dtype=np.float32)
    assert u.shape == (NX, NY), u.shape
    D = float(np.asarray(D_eff).reshape(-1)[0])
    st = np.asarray(stencil).reshape(-1)
    s0, s1 = float(st[0]), float(st[1])
    dv = np.asarray(dirichlet_val, dtype=np.float32).reshape(-1)
    a = np.float32(D * s0)
    c = np.float32(4.0 * D * s1)

    S = np.empty((NX + 2, NY + 2), dtype=np.float32)
    S[1:-1, 1:-1] = u
    S[0, :] = dv[0]       # x- boundary (row 0 up-neighbor)
    S[-1, :] = dv[1]      # x+ boundary
    S[1:-1, 0] = dv[2]    # y- boundary
    S[1:-1, -1] = dv[3]   # y+ boundary

    w_main = _tridiag(128, a, c)
    w_last = _tridiag(LAST_IN, a, c)
    coef = np.full((128, 1), a, dtype=np.float32)

    in_maps = [
        {
            "s_in": np.ascontiguousarray(S[ROWS_PER_CORE * k : ROWS_PER_CORE * k + SLAB_ROWS]),
            "w_main": w_main,
            "w_last": w_last,
            "coef": coef,
        }
        for k in range(N_CORES)
    ]

    from concourse.bass_utils import run_bass_kernel_spmd

    res = run_bass_kernel_spmd(_get_nc(), in_maps, core_ids=list(range(N_CORES)))
    return np.concatenate([r["out"] for r in res.results], axis=0)
